# revision 41
# baseline (speedup 1.0000x reference)
"""MoE layer (top-2 of 8 experts) on 8 TRN2 NeuronCores, expert-parallel.

Host side: router (exact replica of the reference jax ops, so top-k
selection bit-matches), token gather by expert assignment, weight
repacking into DMA-friendly bf16 layouts, and the final weighted
scatter-add in fp32.

Device side (one expert per core, SPMD): the full expert FFN
    h = X @ W1 ; act = gelu(h_gate) * h_up ; Y = act @ W2
computed with bf16 matmul operands (full-rate on the PE at any free
size, FWL-eligible weight loads, half the DMA bytes of fp32) and fp32
PSUM accumulation.  All activations stay transposed (tokens on the
free axis) so no on-device transposes are needed.

Self-contained: only library imports (numpy/jax/ml_dtypes/concourse).
"""

import numpy as np

TOP_K = 2
EPS = 1e-6
P = 128
D = 2048
F = 2048  # expert hidden dim (ED)
E = 8
KO = D // P  # 16 K-tiles for matmul1 / output D-tiles
MJ = F // P  # 16 gate/up strip pairs; also K-tiles for matmul2

_BUILD_CACHE: dict = {}

# Activation for the gate branch. CoreSim doesn't implement Gelu, so tests
# can set this to "Identity" for structural sim validation.
ACT_FN = "Gelu"


def _chunks_of(C: int) -> list[tuple[int, int]]:
    """Split the token-capacity free axis into matmul chunks <= 504,
    roughly equal, multiples of 8 (just under the 512-fp32 PSUM bank)."""
    nch = -(-C // 504)
    base = C // nch
    base -= base % 8
    sizes = [base] * nch
    rem = C - base * nch
    i = 0
    while rem > 0:
        add = min(8, rem)
        sizes[i % nch] += add
        rem -= add
        i += 1
    out = []
    off = 0
    for s in sizes:
        out.append((off, s))
        off += s
    assert off == C
    return out


def _build(C: int):
    """Build + compile the per-core expert-FFN bass program for capacity C."""
    key = (C, ACT_FN)
    if key in _BUILD_CACHE:
        return _BUILD_CACHE[key]

    import concourse.bacc as bacc
    import concourse.mybir as mybir
    import concourse.tile as tile
    f32 = mybir.dt.float32
    bf16 = mybir.dt.bfloat16
    act_fn = getattr(mybir.ActivationFunctionType, ACT_FN)
    chunks = _chunks_of(C)
    nch = len(chunks)

    nc = bacc.Bacc(
        "TRN2", target_bir_lowering=False, debug=False, enable_asserts=False
    )
    # Packed layouts (host pre-transposed, partition-major, all bf16):
    #   xt{c}[p, ko, t]  = X^T[ko*128+p, c0+t]        (chunk-blocked tokens)
    #   w1[p, m, ko, q]  = W1perm[ko*128+p, m*128+q]  (m: g0,u0,g1,u1,... strips)
    #   w2[p, i, fo, q]  = W2[fo*128+p, i*128+q]
    #   yt[p, io, t]     = Y^T[io*128+p, t]
    xt_d = [
        nc.dram_tensor(f"xt{c}", [P, KO, cn], bf16, kind="ExternalInput")
        for c, (c0, cn) in enumerate(chunks)
    ]
    w1_d = nc.dram_tensor("w1", [P, 2 * MJ, KO, P], bf16, kind="ExternalInput")
    w2_d = nc.dram_tensor("w2", [P, KO, MJ, P], bf16, kind="ExternalInput")
    yt_d = nc.dram_tensor("yt", [P, KO, C], bf16, kind="ExternalOutput")

    with tile.TileContext(nc) as tc:
        with (
            tc.tile_pool(name="xt", bufs=1) as xt_pool,
            tc.tile_pool(name="act", bufs=1) as act_pool,
            tc.tile_pool(name="w1", bufs=5) as w1_pool,
            tc.tile_pool(name="w2", bufs=3) as w2_pool,
            tc.tile_pool(name="tg", bufs=3) as tg_pool,
            tc.tile_pool(name="yo", bufs=3) as yo_pool,
            tc.tile_pool(name="ps", bufs=8, space="PSUM") as ps_pool,
        ):
            xt_sb = [
                xt_pool.tile([P, KO, cn], bf16, tag=f"xt{c}", name=f"xt_sb{c}")
                for c, (c0, cn) in enumerate(chunks)
            ]
            act_sb = act_pool.tile([P, MJ, C], bf16)

            # --- DMA plan -------------------------------------------------
            # Three queues: sync (SP HWDGE), scalar (ACT HWDGE), gpsimd
            # (SWDGE), each FIFO.  Startup streams the first chains'
            # operands in consumption order across all three; steady state
            # alternates w1 pairs between the HWDGE rings (w2 rides along).
            w1_tiles: dict[int, object] = {}

            def issue_w1_pair(j, ring):
                t = w1_pool.tile([P, 2, KO, P], bf16, tag="w1p")
                ring.dma_start(t[:], w1_d.ap()[:, 2 * j : 2 * j + 2])
                w1_tiles[j] = t

            # PE warm-up: a few matmuls on scratch SBUF right after the
            # framework preamble, so the PE is busy while the first real
            # operands stream in (starts the HAM activity window early).
            warm_sb = act_pool.tile([P, 512], bf16, tag="warm")
            warm_ps = ps_pool.tile([P, 512], f32, tag="ps")
            nc.gpsimd.memset(warm_sb[:], 0.0)
            for _ in range(12):
                nc.tensor.matmul(
                    warm_ps[:, :256],
                    warm_sb[:, :128],
                    warm_sb[:, 256:512],
                    start=True,
                    stop=True,
                )

            # startup: early DMA rate is globally limited (~50-110GB/s per
            # queue for the first ~10us), so stream the first chain's
            # operands in consumption order as medium pieces across all
            # three queues (sync HWDGE / scalar HWDGE / gpsimd SWDGE).
            t0 = w1_pool.tile([P, 2, KO, P], bf16, tag="w1p")
            nc.scalar.dma_start(t0[:, 0, 0:2], w1_d.ap()[:, 0, 0:2])
            nc.sync.dma_start(xt_sb[0][:, 0:1], xt_d[0].ap()[:, 0:1])
            nc.gpsimd.dma_start(t0[:, 1, 0:8], w1_d.ap()[:, 1, 0:8])
            nc.scalar.dma_start(t0[:, 0, 2:5], w1_d.ap()[:, 0, 2:5])
            nc.sync.dma_start(xt_sb[0][:, 1:3], xt_d[0].ap()[:, 1:3])
            nc.gpsimd.dma_start(t0[:, 1, 8:16], w1_d.ap()[:, 1, 8:16])
            nc.scalar.dma_start(t0[:, 0, 5:9], w1_d.ap()[:, 0, 5:9])
            nc.sync.dma_start(xt_sb[0][:, 3:6], xt_d[0].ap()[:, 3:6])
            nc.scalar.dma_start(t0[:, 0, 9:16], w1_d.ap()[:, 0, 9:16])
            nc.sync.dma_start(xt_sb[0][:, 6:9], xt_d[0].ap()[:, 6:9])
            nc.sync.dma_start(xt_sb[0][:, 9:12], xt_d[0].ap()[:, 9:12])
            nc.sync.dma_start(xt_sb[0][:, 12:16], xt_d[0].ap()[:, 12:16])
            w1_tiles[0] = t0
            # pairs 1-2 split by strip across the two weight queues
            for j in (1, 2):
                tj = w1_pool.tile([P, 2, KO, P], bf16, tag="w1p", name=f"w1p{j}")
                nc.scalar.dma_start(tj[:, 0], w1_d.ap()[:, 2 * j])
                nc.gpsimd.dma_start(tj[:, 1], w1_d.ap()[:, 2 * j + 1])
                w1_tiles[j] = tj
            for c in range(1, nch):
                nc.sync.dma_start(xt_sb[c][:, 0:8], xt_d[c].ap()[:, 0:8])
                nc.sync.dma_start(xt_sb[c][:, 8:16], xt_d[c].ap()[:, 8:16])

            # ffn1 visit plan: the first `head` strips process chunk 0 only,
            # then revisit their remaining chunks — defers the xt c1/c2
            # demand past the cold-start window so the PE never starves.
            if nch >= 2:
                head = min(3, MJ)
                visit_plan = [(j, [0]) for j in range(head)]
                visit_plan += [(j, list(range(1, nch))) for j in range(head)]
                visit_plan += [(j, list(range(nch))) for j in range(head, MJ)]
            else:
                visit_plan = [(j, [0]) for j in range(MJ)]
            visits_left: dict[int, int] = {}
            for j, _cs in visit_plan:
                visits_left[j] = visits_left.get(j, 0) + 1

            with nc.named_scope("ffn1"):
                next_pair = 3
                for v, (j, cs) in enumerate(visit_plan):
                    # keep pair issue ~2 visits ahead of use
                    for fv in range(v + 1, min(v + 3, len(visit_plan))):
                        jf = visit_plan[fv][0]
                        if jf >= next_pair:
                            while next_pair <= jf:
                                issue_w1_pair(
                                    next_pair,
                                    nc.sync if (next_pair % 2) else nc.scalar,
                                )
                                next_pair += 1
                    wpair = w1_tiles[j]
                    for c in cs:
                        c0, cn = chunks[c]
                        pg = ps_pool.tile([P, 512], f32, tag="ps")
                        pu = ps_pool.tile([P, 512], f32, tag="ps")
                        for ko in range(KO):
                            nc.tensor.matmul(
                                pg[:, :cn],
                                wpair[:, 0, ko],
                                xt_sb[c][:, ko],
                                start=(ko == 0),
                                stop=(ko == KO - 1),
                            )
                        for ko in range(KO):
                            nc.tensor.matmul(
                                pu[:, :cn],
                                wpair[:, 1, ko],
                                xt_sb[c][:, ko],
                                start=(ko == 0),
                                stop=(ko == KO - 1),
                            )
                        tg = tg_pool.tile([P, 512], f32, tag="tg")
                        nc.scalar.activation(tg[:, :cn], pg[:, :cn], act_fn)
                        nc.vector.tensor_mul(
                            out=act_sb[:, j, c0 : c0 + cn],
                            in0=tg[:, :cn],
                            in1=pu[:, :cn],
                        )
                    visits_left[j] -= 1
                    if visits_left[j] == 0:
                        w1_tiles.pop(j)

            with nc.named_scope("ffn2"):
                w2_tiles: dict[int, object] = {}

                def issue_w2_pair(ip, ring):
                    t = w2_pool.tile([P, 2, MJ, P], bf16, tag="w2p")
                    ring.dma_start(t[:], w2_d.ap()[:, 2 * ip : 2 * ip + 2])
                    w2_tiles[ip] = t

                issue_w2_pair(0, nc.sync)
                issue_w2_pair(1, nc.scalar)
                # last strip's final chunk is processed in two halves so the
                # kernel tail (final cast + output DMA) is half as long
                lc0, lcn = chunks[-1]
                h1 = (lcn // 2 + 7) // 8 * 8
                tail_chunks = chunks[:-1] + [(lc0, h1), (lc0 + h1, lcn - h1)]
                for i in range(KO):
                    ip, half = divmod(i, 2)
                    if half == 0:
                        if ip + 2 < KO // 2:
                            issue_w2_pair(ip + 2, nc.sync if (ip % 2) else nc.scalar)
                        w2t = w2_tiles[ip]
                    yo = yo_pool.tile([P, C], bf16, tag="yo")
                    for c0, cn in (chunks if i < KO - 1 else tail_chunks):
                        py = ps_pool.tile([P, 512], f32, tag="ps")
                        for fo in range(MJ):
                            nc.tensor.matmul(
                                py[:, :cn],
                                w2t[:, half, fo],
                                act_sb[:, fo, c0 : c0 + cn],
                                start=(fo == 0),
                                stop=(fo == MJ - 1),
                            )
                        nc.vector.tensor_copy(
                            out=yo[:, c0 : c0 + cn], in_=py[:, :cn]
                        )
                        if i == KO - 1:
                            # stream the final strip per chunk to shorten
                            # the kernel tail
                            nc.sync.dma_start(
                                yt_d.ap()[:, i, c0 : c0 + cn],
                                yo[:, c0 : c0 + cn],
                            )
                    if half == 1:
                        w2_tiles.pop(ip)
                    if i < KO - 1:
                        nc.sync.dma_start(yt_d.ap()[:, i], yo[:])

    nc.compile()
    _BUILD_CACHE[key] = nc
    return nc


def _router(x, router_scale, gate_w):
    """Replicate the reference router ops exactly (same jax ops, default
    backend) so the top-2 expert selection bit-matches the reference."""
    import jax
    import jax.numpy as jnp

    x = jnp.asarray(x)
    router_scale = jnp.asarray(router_scale)
    gate_w = jnp.asarray(gate_w)
    _B, _L, d = x.shape
    h = x * jax.lax.rsqrt(jnp.mean(x * x, axis=-1, keepdims=True) + EPS)
    h = h * (d**-0.5) * router_scale
    logits = (h @ gate_w).astype(jnp.float32)
    probs = jax.nn.softmax(logits, axis=-1)
    w, idx = jax.lax.top_k(probs, TOP_K)
    w = w / jnp.clip(jnp.sum(w, axis=-1, keepdims=True), 1e-12)
    w = w.astype(x.dtype)
    return (
        np.asarray(idx).reshape(-1, TOP_K),
        np.asarray(w).reshape(-1, TOP_K).astype(np.float32),
    )


def _bf16(a: np.ndarray) -> np.ndarray:
    import ml_dtypes

    return np.ascontiguousarray(a).astype(ml_dtypes.bfloat16)


def _pack_w1(gate_up_e: np.ndarray) -> np.ndarray:
    """[D, 2F] -> [P, 2*MJ, KO, P] bf16 with gate/up 128-col strips
    interleaved."""
    g = gate_up_e[:, :F].reshape(D, MJ, P)
    u = gate_up_e[:, F:].reshape(D, MJ, P)
    w1p = np.empty((D, 2 * MJ, P), np.float32)
    w1p[:, 0::2] = g
    w1p[:, 1::2] = u
    # [D, 2MJ, P] -> [KO, P, 2MJ, P] -> [P, 2MJ, KO, P]
    return _bf16(w1p.reshape(KO, P, 2 * MJ, P).transpose(1, 2, 0, 3))


def _pack_w2(down_e: np.ndarray) -> np.ndarray:
    """[F, D] -> [P, KO, MJ, P] bf16 (w2[p, i, fo, q] = W2[fo*128+p, i*128+q])."""
    return _bf16(down_e.reshape(MJ, P, KO, P).transpose(1, 2, 0, 3))


def run_moe(x, router_scale, gate_w, gate_up, down, per_expert_scale, trace=False):
    from concourse import bass_utils

    x = np.asarray(x, dtype=np.float32)
    router_scale = np.asarray(router_scale, dtype=np.float32)
    gate_w = np.asarray(gate_w, dtype=np.float32)
    gate_up = np.asarray(gate_up, dtype=np.float32)
    down = np.asarray(down, dtype=np.float32)
    per_expert_scale = np.asarray(per_expert_scale, dtype=np.float32)

    B, L, d = x.shape
    N = B * L
    assert d == D and gate_up.shape == (E, D, 2 * F) and down.shape == (E, F, D)

    idxf, wf = _router(x, router_scale, gate_w)

    pair_expert = idxf.reshape(-1)
    pair_token = np.repeat(np.arange(N), TOP_K)
    pair_w = wf.reshape(-1) * per_expert_scale[pair_expert]

    order = np.argsort(pair_expert, kind="stable")
    tok_o = pair_token[order]
    w_o = pair_w[order]
    counts = np.bincount(pair_expert, minlength=E)
    offs = np.zeros(E + 1, np.int64)
    offs[1:] = np.cumsum(counts)

    # Capacity choice: the device computes the first C pairs of each
    # expert's token list; a small overflow above C (sub-percent for
    # near-balanced routing) is computed on host in exact fp32 so that all
    # cores run an identical, perfectly balanced program.  Extreme routing
    # imbalance falls back to multiple launches over row segments.
    cmax_count = int(counts.max())
    C_bal = max(64, -(-(-(-len(tok_o) // E)) // 8) * 8)  # ceil(total/E) pad8
    # cap at 2 full near-bank-width chunks; the small extra overflow goes
    # to the host fallback below
    C_bal = min(C_bal, 1008)
    spill = int(np.maximum(counts - C_bal, 0).sum())
    if cmax_count <= C_bal or spill <= max(64, len(tok_o) // 25):
        C = min(C_bal, -(-cmax_count // 8) * 8)
        nseg = 1
    else:
        CMAX = 1344
        nseg = max(1, -(-cmax_count // CMAX))
        seg_cap = -(-cmax_count // nseg)
        C = max(64, -(-seg_cap // 8) * 8)
    chunks = _chunks_of(C)

    nc = _build(C)

    xf = x.reshape(N, D)
    w1_packed = [_pack_w1(gate_up[e]) for e in range(E)]
    w2_packed = [_pack_w2(down[e]) for e in range(E)]

    contrib = np.empty((len(tok_o), D), np.float32)
    res = None
    for s in range(nseg):
        in_maps = []
        ranges = []
        for e in range(E):
            lo = min(offs[e] + s * C, offs[e + 1])
            hi = min(lo + C, offs[e + 1])
            toks = tok_o[lo:hi]
            ranges.append((lo, hi))
            xg = np.zeros((C, D), np.float32)
            xg[: len(toks)] = xf[toks]
            xt = _bf16(xg.T.reshape(KO, P, C).transpose(1, 0, 2))
            im = {"w1": w1_packed[e], "w2": w2_packed[e]}
            for c, (c0, cn) in enumerate(chunks):
                im[f"xt{c}"] = np.ascontiguousarray(xt[:, :, c0 : c0 + cn])
            in_maps.append(im)

        res = bass_utils.run_bass_kernel_spmd(
            nc, in_maps, core_ids=list(range(E)), trace=trace and s == 0
        )
        for e in range(E):
            lo, hi = ranges[e]
            yt = np.asarray(res.results[e]["yt"]).astype(np.float32)  # [P, KO, C]
            ytd = yt.transpose(1, 0, 2).reshape(D, C)  # [D, C]
            contrib[lo:hi] = ytd[:, : hi - lo].T

    # Host fallback for expert-capacity overflow (exact fp32 FFN on the few
    # pairs beyond nseg*C of an expert's list; zero rows for balanced C).
    for e in range(E):
        lo = int(min(offs[e] + nseg * C, offs[e + 1]))
        hi = int(offs[e + 1])
        if lo >= hi:
            continue
        try:
            from scipy.special import erf
        except ImportError:
            import math

            erf = np.frompyfunc(math.erf, 1, 1)

        xs = xf[tok_o[lo:hi]]
        h = xs @ gate_up[e]
        g, u = h[:, :F], h[:, F:]
        act = (0.5 * g * (1.0 + erf(g * np.float32(2.0**-0.5)))).astype(
            np.float32
        ) * u
        contrib[lo:hi] = act @ down[e]

    contrib *= w_o[:, None]

    s = np.argsort(tok_o, kind="stable")
    tok_s = tok_o[s]
    out = np.zeros((N, D), np.float32)
    if len(tok_s) == 2 * N and np.array_equal(tok_s[0::2], tok_s[1::2]):
        cs = contrib[s]
        out[tok_s[0::2]] = cs[0::2] + cs[1::2]
    else:  # defensive fallback (duplicate experts per token can't happen)
        np.add.at(out, tok_o, contrib)
    return out.reshape(B, L, D), res


def kernel(x, router_scale, gate_w, gate_up, down, per_expert_scale):
    out, _ = run_moe(x, router_scale, gate_w, gate_up, down, per_expert_scale)
    return out
